# revision 3
# baseline (speedup 1.0000x reference)
#!/usr/bin/env python
"""Trainium2 Bass kernel for nn_Continuous_Tucker (SIREN x3 + Tucker core).

Data-parallel over the batch across 8 NeuronCores. Each core computes, for
its 8192-element batch slice:
  U/V/W = siren(x_i) for the three 1->512->512->32 nets with
          sine_layer(x) = sin(sin(4*(x @ W.T + b)))
  out[b] = sum_{r,s,t} U[b,r] V[b,s] W[b,t] C[r,s,t]

Key device-side design:
- Everything runs features-on-partitions; batch along the free dim.
- ACT's Sin spline is only valid on ~[-3.555, 3.555]. Layer-1 args reach
  +-8, so layer 1 works in "turns": f = w'*x_c + c'' with w' = 2*w1/pi,
  x_c = x - 0.5, and the per-feature phase c'' host-folded into [-1/4, 1/4]
  (mod 1, with a per-feature half-turn shift whose sign flip is absorbed
  into the next layer's weight columns). Then |2*pi*f| <= ~3.57 and
  sin(2*pi*f) = sin(4*(w1*x + b1)) exactly. One DVE op per chunk.
- Layer-2 args are bounded by 4*(0.8415*max_row_sum|w2| + max|b2|) < 3.55
  for these weight fills (asserted on host).
- Matmuls run in float32r (1 cycle/row vs 4 for fp32; ~1.6e-4 rel err).
- Tucker contraction: K2[(s,t), b] = V[s,b]*W[t,b] built via pattern
  replication matmuls + DVE products, then T2 = C3.T @ K2 accumulated in
  PSUM over 8 K-chunks, final dot with U via DVE + ones-matmul reduce.
"""
import sys

for _p in ("/opt/trn_rl_repo", "/root/.axon_site/_ro/trn_rl_repo"):
    if _p not in sys.path:
        sys.path.insert(0, _p)

import numpy as np

import concourse.bass as bass
import concourse.mybir as mybir
import concourse.tile as tile
from concourse import bacc
from concourse.bass_utils import run_bass_kernel_spmd

f32 = mybir.dt.float32
f32r = mybir.dt.float32r
AF = mybir.ActivationFunctionType
OP = mybir.AluOpType

N_CORES = 8
B = 65536
B_CORE = B // N_CORES
SUPER = 2048
NSUP = B_CORE // SUPER
NS = SUPER // 512  # 512-wide subtiles per super-tile
MID = 512
R = 32
OMEGA = 4.0
TWO_PI = float(2.0 * np.pi)

_CACHE = {}


def _build_body(nc, tc, d, out):
    with (
        tc.tile_pool(name="const", bufs=1) as const,
        tc.tile_pool(name="acts", bufs=1) as acts,
        tc.tile_pool(name="work", bufs=2) as work,
        tc.tile_pool(name="ps_l2", bufs=2, space="PSUM") as ps_l2,
        tc.tile_pool(name="ps_med", bufs=3, space="PSUM") as ps_med,
        tc.tile_pool(name="ps_sm", bufs=1, space="PSUM") as ps_sm,
    ):
        _body_inner(
            nc, tc, d, out, const, acts, work, ps_l2, ps_med, ps_sm
        )


def _body_inner(nc, tc, d, out, const, acts, work, ps_l2, ps_med, ps_sm):
    # ---- constants into SBUF
    w2sb = [
        [const.tile([128, MID], f32r, name=f"w2sb_{n}_{k}") for k in range(4)]
        for n in range(3)
    ]
    w3sb = [
        [const.tile([128, R], f32r, name=f"w3sb_{n}_{k}") for k in range(4)]
        for n in range(3)
    ]
    b2sb, b3sb, wpsb, c2sb = [], [], [], []
    for n in range(3):
        for k in range(4):
            nc.gpsimd.dma_start(out=w2sb[n][k], in_=d["w2t"].ap()[n, k])
            nc.gpsimd.dma_start(out=w3sb[n][k], in_=d["w3t"].ap()[n, k])
        t = const.tile([128, 4], f32, name=f"b2sb_{n}")
        nc.sync.dma_start(out=t, in_=d["b2c"].ap()[n])
        b2sb.append(t)
        t = const.tile([R, 1], f32, name=f"b3sb_{n}")
        nc.sync.dma_start(out=t, in_=d["b3c"].ap()[n])
        b3sb.append(t)
        t = const.tile([128, 4], f32, name=f"wpsb_{n}")
        nc.sync.dma_start(out=t, in_=d["wpc"].ap()[n])
        wpsb.append(t)
        t = const.tile([128, 4], f32, name=f"c2sb_{n}")
        nc.sync.dma_start(out=t, in_=d["c2c"].ap()[n])
        c2sb.append(t)
    pwsb = const.tile([R, 128], f32r, name="pwsb")
    nc.gpsimd.dma_start(out=pwsb, in_=d["pw"].ap())
    pvsb = [const.tile([R, 128], f32r, name=f"pvsb_{c}") for c in range(8)]
    c3sb = [const.tile([128, R], f32r, name=f"c3sb_{c}") for c in range(8)]
    for c in range(8):
        nc.gpsimd.dma_start(out=pvsb[c], in_=d["pv"].ap()[c])
        nc.gpsimd.dma_start(out=c3sb[c], in_=d["c3"].ap()[c])
    onesb = const.tile([R, 1], f32r, name="onesb")
    nc.gpsimd.dma_start(out=onesb, in_=d["ones_r"].ap())

    out2d = out.ap().rearrange("(a b) -> a b", a=NSUP * NS)
    x_ap = d["xc"].ap()

    for st in range(NSUP):
        uvw = []
        for n in range(3):
            # ---- x broadcast to 128 partitions (x already centered on host)
            xbc = work.tile([128, SUPER], f32, name="xbc", tag="xbc", bufs=2)
            src = bass.AP(
                tensor=x_ap.tensor,
                offset=n * B_CORE + st * SUPER,
                ap=[[0, 128], [1, SUPER]],
            )
            nc.gpsimd.dma_start(out=xbc, in_=src)

            # ---- layer 1 in turns: f = w'*x_c + c''  (|f| <= ~0.568)
            ftile = acts.tile([128, 4, SUPER], f32, name="ftile", tag="f")
            for m in range(4):
                nc.vector.tensor_scalar(
                    ftile[:, m, :],
                    xbc,
                    wpsb[n][:, m : m + 1],
                    c2sb[n][:, m : m + 1],
                    OP.mult,
                    OP.add,
                )
            # s1 = sin(2*pi*f) == sin(4*(w1*x+b1)) (up to absorbed sign)
            nc.scalar.activation(ftile, ftile, AF.Sin, scale=TWO_PI)
            h1 = acts.tile([128, 4, SUPER], f32r, name="h1", tag="h1")
            nc.scalar.activation(h1, ftile, AF.Sin)

            # ---- layer 2: z2 = w2_eff @ h1 + b2 ; h2 = sin(sin(4*z2))
            h2 = acts.tile([128, 4, SUPER], f32r, name="h2", tag="h2")
            for m in range(4):
                for ns in range(NS):
                    pt = ps_l2.tile([128, 512], f32, name="l2ps", tag="l2")
                    for k in range(4):
                        nc.tensor.matmul(
                            pt,
                            lhsT=w2sb[n][k][:, m * 128 : (m + 1) * 128],
                            rhs=h1[:, k, ns * 512 : (ns + 1) * 512],
                            start=(k == 0),
                            stop=(k == 3),
                        )
                    nc.vector.tensor_scalar_add(
                        h2[:, m, ns * 512 : (ns + 1) * 512],
                        pt,
                        b2sb[n][:, m : m + 1],
                    )
            nc.scalar.activation(h2, h2, AF.Sin, scale=OMEGA)
            nc.scalar.activation(h2, h2, AF.Sin)

            # ---- layer 3: (32, SUPER) = w3 @ h2 + b3
            uv = acts.tile(
                [R, SUPER],
                f32 if n == 0 else f32r,
                name=f"uvw{n}",
                tag=f"uvw{n}",
                bufs=1,
            )
            for ns in range(NS):
                pt3 = ps_sm.tile([R, 512], f32, name="l3ps", tag="l3")
                for k in range(4):
                    nc.tensor.matmul(
                        pt3,
                        lhsT=w3sb[n][k],
                        rhs=h2[:, k, ns * 512 : (ns + 1) * 512],
                        start=(k == 0),
                        stop=(k == 3),
                    )
                nc.vector.tensor_scalar_add(
                    uv[:, ns * 512 : (ns + 1) * 512], pt3, b3sb[n]
                )
            uvw.append(uv)

        # ---- Tucker contraction
        U, V, W = uvw
        for ns in range(NS):
            nsl = slice(ns * 512, (ns + 1) * 512)
            ptw = ps_med.tile([128, 512], f32, name="wrep_ps", tag="med")
            nc.tensor.matmul(ptw, lhsT=pwsb, rhs=W[:, nsl], start=True, stop=True)
            wrep = work.tile([128, 512], f32, name="wrep", tag="wrep", bufs=2)
            nc.vector.tensor_copy(wrep, ptw)
            t2 = ps_sm.tile([R, 512], f32, name="t2ps", tag="t2")
            for c in range(8):
                ptv = ps_med.tile([128, 512], f32, name="vrep_ps", tag="med")
                nc.tensor.matmul(
                    ptv, lhsT=pvsb[c], rhs=V[:, nsl], start=True, stop=True
                )
                k2 = work.tile([128, 512], f32r, name="k2", tag="k2", bufs=3)
                nc.vector.tensor_mul(k2, ptv, wrep)
                nc.tensor.matmul(
                    t2, lhsT=c3sb[c], rhs=k2, start=(c == 0), stop=(c == 7)
                )
            m3 = work.tile([R, 512], f32r, name="m3", tag="m3", bufs=2)
            nc.vector.tensor_mul(m3, t2, U[:, nsl])
            pto = ps_sm.tile([1, 512], f32, name="orow_ps", tag="orow_ps")
            nc.tensor.matmul(pto, lhsT=onesb, rhs=m3, start=True, stop=True)
            orow = work.tile([1, 512], f32, name="orow", tag="orow", bufs=2)
            nc.vector.tensor_copy(orow, pto)
            nc.sync.dma_start(
                out=out2d[st * NS + ns : st * NS + ns + 1, :], in_=orow
            )


def _build_nc():
    nc = bacc.Bacc(
        "TRN2", target_bir_lowering=False, debug=False, num_devices=N_CORES
    )
    d = {}
    for name, shape in (
        ("xc", (3, B_CORE)),
        ("w2t", (3, 4, 128, MID)),
        ("b2c", (3, 128, 4)),
        ("w3t", (3, 4, 128, R)),
        ("b3c", (3, R, 1)),
        ("wpc", (3, 128, 4)),
        ("c2c", (3, 128, 4)),
        ("pw", (R, 128)),
        ("pv", (8, R, 128)),
        ("c3", (8, 128, R)),
        ("ones_r", (R, 1)),
    ):
        d[name] = nc.dram_tensor(name, shape, f32, kind="ExternalInput")
    out = nc.dram_tensor("out", (B_CORE,), f32, kind="ExternalOutput")
    with tile.TileContext(nc) as tc:
        _build_body(nc, tc, d, out)
    nc.compile()
    return nc


def prep_weights(inputs):
    """Host-side packing of all weight-derived device inputs (core-independent)."""
    w = {}
    ww = {k: np.asarray(v, np.float32) for k, v in inputs.items()}
    w2t = np.empty((3, 4, 128, MID), np.float32)
    b2c = np.empty((3, 128, 4), np.float32)
    w3t = np.empty((3, 4, 128, R), np.float32)
    b3c = np.empty((3, R, 1), np.float32)
    wpc = np.empty((3, 128, 4), np.float32)
    c2c = np.empty((3, 128, 4), np.float32)
    for n, pfx in enumerate(("U", "V", "W")):
        w1 = ww[pfx + "w1"][:, 0]  # (512,)
        b1 = ww[pfx + "b1"]
        w2 = ww[pfx + "w2"]
        b2 = ww[pfx + "b2"]
        w3 = ww[pfx + "w3"]
        b3 = ww[pfx + "b3"]
        # layer-2 arg domain check (ACT sin valid |arg| <= ~3.555)
        bound = OMEGA * (
            np.sin(1.0) * np.abs(w2).sum(axis=1).max() + np.abs(b2).max()
        )
        assert bound < 3.55, f"layer-2 sin arg bound {bound} exceeds ACT domain"
        # layer-1 turns: f = w'*(x-0.5) + c'' ; sign flips into w2 columns
        wp = np.float64(2.0 / np.pi) * w1.astype(np.float64)  # 4*w1/(2pi)
        c0 = np.float64(2.0 / np.pi) * b1.astype(np.float64) + 0.5 * wp
        c1 = c0 - np.round(c0)
        flip = np.abs(c1) > 0.25
        c2 = np.where(flip, c1 - 0.5 * np.sign(c1), c1)
        F = np.where(flip, -1.0, 1.0)
        w2_eff = (w2.astype(np.float64) * F[None, :]).astype(np.float32)
        w2t[n] = w2_eff.T.reshape(4, 128, MID)
        b2c[n] = b2.reshape(4, 128).T
        w3t[n] = w3.T.reshape(4, 128, R).astype(np.float32)
        b3c[n] = b3.reshape(R, 1)
        wpc[n] = wp.astype(np.float32).reshape(4, 128).T
        c2c[n] = c2.astype(np.float32).reshape(4, 128).T
    w["w2t"], w["b2c"], w["w3t"], w["b3c"] = w2t, b2c, w3t, b3c
    w["wpc"], w["c2c"] = wpc, c2c
    # Tucker patterns and matricized core
    q = np.arange(128)
    pw = (q[None, :] % R == np.arange(R)[:, None]).astype(np.float32)
    pv = np.zeros((8, R, 128), np.float32)
    c3 = np.empty((8, 128, R), np.float32)
    C = ww["core"].reshape(R, R, R)
    for c in range(8):
        s = 4 * c + q // 32
        pv[c][s, q] = 1.0
        c3[c] = C[:, s, q % 32].T
    w["pw"], w["pv"], w["c3"] = pw, pv, c3
    w["ones_r"] = np.ones((R, 1), np.float32)
    return w


def make_in_maps(inputs):
    w = prep_weights(inputs)
    x = np.asarray(inputs["train_ind_batch"], np.float32)
    in_maps = []
    for c in range(N_CORES):
        sl = x[c * B_CORE : (c + 1) * B_CORE]
        m = dict(w)
        m["xc"] = np.ascontiguousarray(sl.T) - 0.5
        in_maps.append(m)
    return in_maps


def get_nc():
    if "nc" not in _CACHE:
        _CACHE["nc"] = _build_nc()
    return _CACHE["nc"]


def kernel(**inputs) -> np.ndarray:
    nc = get_nc()
    in_maps = make_in_maps(inputs)
    res = run_bass_kernel_spmd(nc, in_maps, core_ids=list(range(N_CORES)))
    return np.concatenate(
        [res.results[c]["out"] for c in range(N_CORES)]
    ).astype(np.float32)


if __name__ == "__main__":
    rng = np.random.default_rng(0)
    # quick self-exercise with random data
    demo = {"train_ind_batch": rng.uniform(0, 1, (B, 3)).astype(np.float32)}
    for pfx in ("U", "V", "W"):
        demo[pfx + "w1"] = rng.uniform(-1, 1, (MID, 1)).astype(np.float32)
        demo[pfx + "b1"] = rng.uniform(-1, 1, MID).astype(np.float32)
        demo[pfx + "w2"] = rng.uniform(-1 / MID, 1 / MID, (MID, MID)).astype(
            np.float32
        )
        demo[pfx + "b2"] = rng.uniform(
            -1 / np.sqrt(MID), 1 / np.sqrt(MID), MID
        ).astype(np.float32)
        demo[pfx + "w3"] = rng.uniform(
            -1 / np.sqrt(MID), 1 / np.sqrt(MID), (R, MID)
        ).astype(np.float32)
        demo[pfx + "b3"] = rng.uniform(
            -1 / np.sqrt(MID), 1 / np.sqrt(MID), R
        ).astype(np.float32)
    demo["core"] = rng.standard_normal(R * R * R).astype(np.float32)
    out = kernel(**demo)
    print("out", out.shape, out[:4])
